# revision 6
# baseline (speedup 1.0000x reference)
"""Trainium2 Bass kernel for nn_MultiHeadAttention_51668456571097.

Computes, for B=16, L=512, LA=8, D=768, H=12, DK=64:
  q = split_heads(query @ Wq + bq), k = split_heads(key @ Wk + bk)
  a = split_heads(aspect @ Wa)
  aspect_scores = sigmoid((einsum('bhad,bhld->bhal', a, k) + bias) * aspect_mask)
  s_attn = softmax(where(mask==0, -1e9, einsum('bhqd,bhkd->bhqk', q, k)/8))
Returns (aspect_scores, s_attn) as the reference does.

Sharding: data-parallel over batch — 2 batches per NeuronCore x 8 cores.
Each core runs an identical Bass/Tile program on its batch slice; weights
are replicated. Host gathers by concatenation along batch.

Per-core dataflow (all fp32):
  - query/key DMA'd in natural [qi, d] layout, transposed on PE (identity
    matmul) to [d, qi] since the PE contracts over partitions.
  - mask int32 is cast to f32 during the SWDGE DMA (gpsimd cast-DMA).
  - projections: stationary = W chunks [din,dout], moving = x^T -> x^T W
    in [dout, qi] layout; per-partition bias folded into the ACT PSUM->SBUF
    copy.
  - scores: per head pair, stationary = qT [64,128] chunks at row groups
    0/64 (concurrent via tile_position), moving = kT [64,512].
  - softmax: ACT exp (scale=1/8) from PSUM, DVE tensor_tensor_reduce
    (exp * maskf -> masked exp + row sums), DVE reciprocal, DVE
    tensor_scalar normalize. Masked entries are exactly 0, matching the
    reference's exp(-1e9 - max) == 0.
  - aspect: sigmoid(z) computed as 0.5*tanh(z/2)+0.5 so ACT stays on the
    exp_and_others table set (no ~2.7us table switch); the (raw+bias)*am
    affine folds into tanh's per-partition scale/bias.
"""

import sys

if "/opt/trn_rl_repo" not in sys.path:
    sys.path.insert(0, "/opt/trn_rl_repo")

import numpy as np

import concourse.bass as bass
import concourse.mybir as mybir
import concourse.tile as tile
from concourse.bass_utils import run_bass_kernel_spmd
from concourse.masks import make_identity

F32 = mybir.dt.float32
I32 = mybir.dt.int32
AF = mybir.ActivationFunctionType
OP = mybir.AluOpType

N_CORES = 8
B, L, LA, D, H = 16, 512, 8, 768, 12
DK = D // H  # 64
BL = B // N_CORES  # batches per core = 2
NT = D // 128  # 6 dout/din tiles
NCH = L // 128  # 4 qi chunks

MAX_WAITS = 1  # this walrus build rejects >1 sem-wait per instruction


def _split_sync_waits(nc: bass.Bass, max_waits: int = MAX_WAITS):
    """Hoist excess sem-waits onto NoOps (same engine, just before the
    instruction) so no instruction carries more than max_waits waits."""
    for fn in nc.m.functions:
        for bb in fn.blocks:
            new_insts = []
            for inst in bb.instructions:
                si = inst.sync_info
                if si is not None and si.on_wait and len(si.on_wait) > max_waits:
                    waits = list(si.on_wait)
                    extra, keep = waits[:-max_waits], waits[-max_waits:]
                    for i in range(0, len(extra), max_waits):
                        nop = mybir.InstNoOp(
                            name=nc.get_next_instruction_name(), ins=[], outs=[]
                        )
                        nop.engine = inst.engine
                        nop.sync_info = mybir.SyncInfo(
                            on_wait=extra[i : i + max_waits], on_update=[]
                        )
                        nc.register_instruction(nop)
                        new_insts.append(nop)
                    inst.sync_info = mybir.SyncInfo(
                        on_wait=keep, on_update=list(si.on_update)
                    )
                new_insts.append(inst)
            bb.instructions[:] = new_insts


def build_program() -> bass.Bass:
    nc = bass.Bass("TRN2")

    t_query = nc.dram_tensor("query", [BL, L, D], F32, kind="ExternalInput")
    t_key = nc.dram_tensor("key", [BL, L, D], F32, kind="ExternalInput")
    t_mask = nc.dram_tensor("mask", [BL, L, L], I32, kind="ExternalInput")
    t_aspect = nc.dram_tensor("aspect", [BL, LA, D], F32, kind="ExternalInput")
    t_wq = nc.dram_tensor("wq", [D, D], F32, kind="ExternalInput")
    t_wk = nc.dram_tensor("wk", [D, D], F32, kind="ExternalInput")
    t_wa = nc.dram_tensor("wa", [D, D], F32, kind="ExternalInput")
    # bqk[p, t, 0] = b_q[t*128+p]; [.., 1] = b_k[t*128+p]
    t_bqk = nc.dram_tensor("bqk", [128, NT, 2], F32, kind="ExternalInput")
    # amsb[b, p, 0] = tanh scale, [b, p, 1] = tanh bias (aspect sigmoid affine)
    t_amsb = nc.dram_tensor("amsb", [BL, 128, 2], F32, kind="ExternalInput")

    t_attn = nc.dram_tensor("attn", [BL, H, L, L], F32, kind="ExternalOutput")
    t_ascore = nc.dram_tensor("ascore", [BL, H, LA, L], F32, kind="ExternalOutput")

    q_dram = t_query[:].rearrange("b (c p) d -> b p c d", p=128)
    k_dram = t_key[:].rearrange("b (c p) d -> b p c d", p=128)
    m_dram = t_mask[:].rearrange("b (c p) j -> b p c j", p=128)
    attn_dram = t_attn[:].rearrange("b h (c p) j -> b h p c j", p=128)

    with tile.TileContext(nc) as tc:
        with (
            tc.tile_pool(name="consts", bufs=1) as consts,
            tc.tile_pool(name="raw", bufs=1) as raw,
            tc.tile_pool(name="maskp", bufs=2) as maskp,
            tc.tile_pool(name="trans", bufs=1) as trans,
            tc.tile_pool(name="proj", bufs=1) as proj,
            tc.tile_pool(name="expep", bufs=4) as expep,
            tc.tile_pool(name="expmp", bufs=10) as expmp,
            tc.tile_pool(name="aout", bufs=2) as aoutp,
            tc.tile_pool(name="small", bufs=8) as small,
            tc.tile_pool(name="sigp", bufs=2) as sigp,
            tc.tile_pool(name="tpsum", bufs=2, space="PSUM") as tpsum,
            tc.tile_pool(name="ppsum", bufs=2, space="PSUM") as ppsum,
            tc.tile_pool(name="spsum", bufs=4, space="PSUM") as spsum,
        ):
            ident = consts.tile([128, 128], F32)
            make_identity(nc, ident)

            wq_sb = consts.tile([128, NT, D], F32)
            wk_sb = consts.tile([128, NT, D], F32)
            wa_sb = consts.tile([128, NT, D], F32)
            nc.sync.dma_start(out=wq_sb, in_=t_wq[:].rearrange("(t p) d -> p t d", p=128))
            nc.sync.dma_start(out=wk_sb, in_=t_wk[:].rearrange("(t p) d -> p t d", p=128))
            nc.sync.dma_start(out=wa_sb, in_=t_wa[:].rearrange("(t p) d -> p t d", p=128))
            bqk_sb = consts.tile([128, NT, 2], F32)
            nc.sync.dma_start(out=bqk_sb, in_=t_bqk[:])
            amsb_sb = consts.tile([128, BL, 2], F32)
            nc.sync.dma_start(out=amsb_sb, in_=t_amsb[:].rearrange("b p c -> p b c"))

            for b in range(BL):
                # ---- input DMA ----
                q_raw = raw.tile([128, NCH, D], F32, tag="q_raw")
                k_raw = raw.tile([128, NCH, D], F32, tag="k_raw")
                a_raw = raw.tile([LA, D], F32, tag="a_raw")
                maskf = maskp.tile([128, NCH, L], F32)
                nc.sync.dma_start(out=q_raw, in_=q_dram[b])
                nc.sync.dma_start(out=k_raw, in_=k_dram[b])
                nc.gpsimd.dma_start(out=maskf, in_=m_dram[b])  # int32 -> f32 cast
                nc.sync.dma_start(out=a_raw, in_=t_aspect[:][b])

                # ---- transpose query/key/aspect to [d, pos] ----
                qryT = trans.tile([128, NT, L], F32, tag="qryT")
                keyT = trans.tile([128, NT, L], F32, tag="keyT")
                aspT = trans.tile([128, NT, LA], F32, tag="aspT")
                for src, dst in ((q_raw, qryT), (k_raw, keyT)):
                    for t in range(NT):
                        pt = tpsum.tile([128, 512], F32, tag="tp")
                        for c in range(NCH):
                            nc.tensor.transpose(
                                pt[:, c * 128 : (c + 1) * 128],
                                src[:, c, t * 128 : (t + 1) * 128],
                                ident,
                            )
                        nc.scalar.copy(out=dst[:, t, :], in_=pt)
                pa = tpsum.tile([128, 512], F32, tag="tp")
                for t in range(NT):
                    nc.tensor.transpose(
                        pa[:, t * LA : (t + 1) * LA],
                        a_raw[:, t * 128 : (t + 1) * 128],
                        ident[:LA, :LA],
                    )
                nc.scalar.copy(
                    out=aspT,
                    in_=pa[:, : NT * LA].rearrange("p (t a) -> p t a", a=LA),
                )

                # ---- projections: xT @ W in [dout, pos] layout ----
                qT = proj.tile([128, NT, L], F32, tag="qT")
                kT = proj.tile([128, NT, L], F32, tag="kT")
                aT = proj.tile([128, NT, 32], F32, tag="aT")
                nc.vector.memset(aT, 0.0)
                for t_out in range(NT):
                    for w_sb, xT, dst, bcol in (
                        (wq_sb, qryT, qT, 0),
                        (wk_sb, keyT, kT, 1),
                    ):
                        pp = ppsum.tile([128, 512], F32, tag="pp")
                        for t_in in range(NT):
                            nc.tensor.matmul(
                                pp,
                                lhsT=w_sb[:, t_in, t_out * 128 : (t_out + 1) * 128],
                                rhs=xT[:, t_in, :],
                                start=(t_in == 0),
                                stop=(t_in == NT - 1),
                            )
                        nc.scalar.activation(
                            out=dst[:, t_out, :],
                            in_=pp,
                            func=AF.Identity,
                            bias=bqk_sb[:, t_out, bcol : bcol + 1],
                            scale=1.0,
                        )
                pa2 = ppsum.tile([128, 512], F32, tag="pp")
                for t_out in range(NT):
                    for t_in in range(NT):
                        nc.tensor.matmul(
                            pa2[:, t_out * LA : (t_out + 1) * LA],
                            lhsT=wa_sb[:, t_in, t_out * 128 : (t_out + 1) * 128],
                            rhs=aspT[:, t_in, :],
                            start=(t_in == 0),
                            stop=(t_in == NT - 1),
                        )
                nc.scalar.copy(
                    out=aT[:, :, :LA],
                    in_=pa2[:, : NT * LA].rearrange("p (t a) -> p t a", a=LA),
                )

                # ---- aspect scores: sigmoid((a.k + bias) * am) via tanh ----
                for g in range(H // 4):
                    ps = spsum.tile([128, 512], F32, tag="sp")
                    for j in range(4):
                        h = 4 * g + j
                        t, r = h // 2, 64 * (h % 2)
                        nc.tensor.matmul(
                            ps[32 * j : 32 * j + 32, :],
                            lhsT=aT[r : r + 64, t, :],
                            rhs=kT[r : r + 64, t, :],
                            start=True,
                            stop=True,
                            tile_position=(r, 32 * j),
                        )
                    sg = sigp.tile([128, 512], F32)
                    nc.scalar.activation(
                        out=sg,
                        in_=ps,
                        func=AF.Tanh,
                        scale=amsb_sb[:, b, 0:1],
                        bias=amsb_sb[:, b, 1:2],
                    )
                    nc.vector.tensor_scalar(
                        out=sg, in0=sg, scalar1=0.5, scalar2=0.5,
                        op0=OP.mult, op1=OP.add,
                    )
                    for j in range(4):
                        nc.sync.dma_start(
                            out=t_ascore[:][b, 4 * g + j],
                            in_=sg[32 * j : 32 * j + LA, :],
                        )

                # ---- attention scores + masked softmax, head pairs ----
                for t in range(NT):
                    sums = [
                        small.tile([128, NCH], F32, tag="sums", name=f"sums{b}_{t}_{i}")
                        for i in range(2)
                    ]
                    aouts = [
                        aoutp.tile([128, NCH, L], F32, tag="aout", name=f"aout{b}_{t}_{i}")
                        for i in range(2)
                    ]
                    expms = {}
                    for c in range(NCH):
                        for r_idx in range(2):
                            ps = spsum.tile([128, 512], F32, tag="sp")
                            r = 64 * r_idx
                            nc.tensor.matmul(
                                ps,
                                lhsT=qT[r : r + 64, t, c * 128 : (c + 1) * 128],
                                rhs=kT[r : r + 64, t, :],
                                start=True,
                                stop=True,
                                tile_position=(r, 0),
                            )
                            ee = expep.tile([128, 512], F32)
                            nc.scalar.activation(
                                out=ee, in_=ps, func=AF.Exp, scale=0.125
                            )
                            em = expmp.tile([128, 512], F32)
                            expms[(r_idx, c)] = em
                            # em = (ee * 1.0) * maskf ; accum = row-sum(em)
                            # (tensor_tensor_reduce is unsupported by this
                            # walrus build; scalar_tensor_tensor+accum is the
                            # same DVE cost class)
                            nc.vector.scalar_tensor_tensor(
                                out=em,
                                in0=ee,
                                scalar=1.0,
                                in1=maskf[:, c, :],
                                op0=OP.mult,
                                op1=OP.mult,
                                accum_out=sums[r_idx][:, c : c + 1],
                            )
                    for r_idx in range(2):
                        h = 2 * t + r_idx
                        rc = small.tile([128, NCH], F32, tag="recip")
                        nc.vector.reciprocal(out=rc, in_=sums[r_idx])
                        for c in range(NCH):
                            nc.vector.tensor_scalar_mul(
                                out=aouts[r_idx][:, c, :],
                                in0=expms[(r_idx, c)],
                                scalar1=rc[:, c : c + 1],
                            )
                        nc.sync.dma_start(out=attn_dram[b, h], in_=aouts[r_idx])

    _split_sync_waits(nc)
    return nc


_PROGRAM_CACHE: dict[str, bass.Bass] = {}


def _get_program() -> bass.Bass:
    if "nc" not in _PROGRAM_CACHE:
        _PROGRAM_CACHE["nc"] = build_program()
    return _PROGRAM_CACHE["nc"]


def _make_in_maps(inputs: dict) -> list[dict]:
    query = np.asarray(inputs["query"], np.float32)
    key = np.asarray(inputs["key"], np.float32)
    mask = np.asarray(inputs["mask"], np.int32)
    aspect = np.asarray(inputs["aspect"], np.float32)
    aspect_mask = np.asarray(inputs["aspect_mask"], np.float32)
    W_q_lin = np.asarray(inputs["W_q_lin"], np.float32)
    b_q_lin = np.asarray(inputs["b_q_lin"], np.float32)
    W_k_lin = np.asarray(inputs["W_k_lin"], np.float32)
    b_k_lin = np.asarray(inputs["b_k_lin"], np.float32)
    W_aspect = np.asarray(inputs["W_aspect"], np.float32)
    bias_val = float(np.asarray(inputs["bias"], np.float32).reshape(-1)[0])

    # per-partition bias vectors for the projection copies
    bqk = np.stack(
        [b_q_lin.reshape(NT, 128).T, b_k_lin.reshape(NT, 128).T], axis=-1
    ).astype(np.float32)  # [128, NT, 2]

    # aspect sigmoid via tanh: sigmoid(z) = 0.5*tanh(z/2) + 0.5,
    # z = (raw + bias)*am  ->  tanh arg = raw*(am/2) + bias*am/2
    # partition p of the packed aspect-score tile holds ai = p % 32
    ai_of_p = np.arange(128) % 32
    amsb_full = np.zeros((B, 128, 2), np.float32)
    for bg in range(B):
        am = np.where(ai_of_p < LA, aspect_mask[bg, np.minimum(ai_of_p, LA - 1), 0], 1.0)
        amsb_full[bg, :, 0] = 0.5 * am
        amsb_full[bg, :, 1] = 0.5 * bias_val * am

    in_maps = []
    for i in range(N_CORES):
        sl = slice(i * BL, (i + 1) * BL)
        in_maps.append(
            {
                "query": np.ascontiguousarray(query[sl]),
                "key": np.ascontiguousarray(key[sl]),
                "mask": np.ascontiguousarray(mask[sl]),
                "aspect": np.ascontiguousarray(aspect[sl]),
                "wq": W_q_lin,
                "wk": W_k_lin,
                "wa": W_aspect,
                "bqk": bqk,
                "amsb": np.ascontiguousarray(amsb_full[sl]),
            }
        )
    return in_maps


def kernel(
    query,
    key,
    mask,
    aspect,
    aspect_mask,
    W_q_lin,
    b_q_lin,
    W_k_lin,
    b_k_lin,
    W_aspect,
    bias,
):
    in_maps = _make_in_maps(
        {
            "query": query,
            "key": key,
            "mask": mask,
            "aspect": aspect,
            "aspect_mask": aspect_mask,
            "W_q_lin": W_q_lin,
            "b_q_lin": b_q_lin,
            "W_k_lin": W_k_lin,
            "b_k_lin": b_k_lin,
            "W_aspect": W_aspect,
            "bias": bias,
        }
    )
    nc = _get_program()
    res = run_bass_kernel_spmd(nc, in_maps, list(range(N_CORES)))
    s_attn = np.concatenate([r["attn"] for r in res.results], axis=0)
    aspect_scores = np.concatenate([r["ascore"] for r in res.results], axis=0)
    return aspect_scores, s_attn
